# revision 1
# baseline (speedup 1.0000x reference)
"""Trainium2 Bass kernel for the 4-group sparse-tap 3x3 conv.

Computation (see reference): x (32,128,56,56) f32, weights (32,2048) f32.
Four groups of 32 output channels; group g uses 4 taps CFG[g] of the 3x3
footprint over all 128 input channels. Output (32,128,56,56) f32.

Strategy: pure data-parallel over batch — 4 images per NeuronCore, 8 cores.

Host prep: zero-pad each image to 59x58 (1-pixel conv halo + one extra row
so the last shifted matmul view stays in-bounds), cast to fp16, and lay the
4 images of a core out channel-major ([128 ic, 4*59*58]) so shards DMA with
large contiguous per-partition descriptors.  Weights are rearranged into 16
[ic=128, oc=32] fp16 stationary blocks, one per (group, tap) pair.

Device, per image: for each chunk of 8 output rows, issue 16 column-tiled
matmuls (tile_position=(0,32g)) — group g's 4 taps accumulate into PSUM
partitions 32g..32g+31.  Tap (kr,kc) uses the rhs slice starting at
(8c+kr)*58+kc, which yields all 8 shifted rows in one contiguous view
thanks to the width padding.  The 4 groups' matmuls execute concurrently on
the PE's 32-column sub-arrays, so a chunk costs ~4 matmul streams instead
of 9 (the dense-tap formulation): ~800ns/chunk warm.  fp16 keeps 10
mantissa bits and accumulates in fp32; outputs are stored fp16 (DVE casts
on the PSUM->SBUF copy) and upcast to f32 on the host, halving output HBM
traffic (total DMA 10.0MB -> 6.8MB per core, under the ~358GB/s cap).

DMA plan: three rings (sync Q1, scalar Q10 HWDGE; gpsimd Q0 SWDGE),
~360GB/s HBM cap total.  Two measured ring behaviors drive the layout:
(1) each dma_start pays ~1.5-2.5us of descriptor-generation/startup
latency before data moves, so transfers are few and large, and a ring's
first DMA (whose latency hides in the framework preamble) is reserved for
the earliest-needed data; (2) front-loading every input at once
oversubscribes HBM and starves the piece the PE needs next, so later
inputs are semaphore-gated behind earlier ones.  sync: img0 front rows,
img0 back rows, then out0 (its completion sem gates slot0 reuse).
scalar: weights, then img1, then out1.  gpsimd: img2, img3, then out2 and
all img3 output pieces, streamed per-chunk-group so the post-compute
flush is short.  The framework teardown is a fixed ~8us semaphore sweep
(engine sequencers run at 1.2GHz regardless of the HAM state, so it
cannot be shortened or overlapped with user work).
"""

from contextlib import ExitStack

import numpy as np

import concourse.bass as bass
import concourse.mybir as mybir
from concourse.bass_utils import run_bass_kernel_spmd

CFG = [[1, 2, 4, 5], [2, 3, 5, 6], [4, 5, 7, 8], [5, 6, 8, 9]]

B, C, H, W = 32, 128, 56, 56
NCORES = 8
BPC = B // NCORES            # images per core
HP, WP = H + 3, W + 2        # padded rows (1 top, 1 bottom, 1 overread), cols
XF = HP * WP                 # 3422 padded free elems per image
OF = H * W                   # 3136 output free elems per image
NPSUM = 8                    # psum banks cycled over chunks
RPC = 8                      # output rows per chunk
NCHUNK = H // RPC            # 7
NFREE = RPC * WP             # 464 matmul free dim
F32 = mybir.dt.float32
F16 = mybir.dt.float16
SLOT = [0, 1, 2, 0]          # output slot per image (3 slots)

CW = RPC * W                 # output cols per chunk (448)

# chunk schedule: (image, first output row, rows).  img3's last 8-row chunk
# is split into two 4-row chunks so the final PSUM copy — which gates the
# last output DMA — is half as long.
CHUNKS = [(b, RPC * c, RPC) for b in range(BPC - 1) for c in range(NCHUNK)]
CHUNKS += [(3, RPC * c, RPC) for c in range(NCHUNK - 1)]
CHUNKS += [(3, 48, 4), (3, 52, 4)]


def _build_nc():
    nc = bass.Bass()
    xp = nc.declare_dram_parameter("x", [C, BPC * XF], F16, isOutput=False)
    wp = nc.declare_dram_parameter("w", [C, 16 * 32], F16, isOutput=False)
    op = nc.declare_dram_parameter("out", [BPC, C, OF], F16, isOutput=True)

    with ExitStack() as ctx:
        w_tile = ctx.enter_context(nc.sbuf_tensor("w_tile", [C, 16 * 32], F16))
        xbuf = ctx.enter_context(nc.sbuf_tensor("xbuf", [C, BPC * XF], F16))
        o_slots = [ctx.enter_context(nc.sbuf_tensor(f"o_slot{i}", [C, OF], F16))
                   for i in range(3)]
        psums = [ctx.enter_context(nc.psum_tensor(f"psum{i}", [C, 512], F32))
                 for i in range(NPSUM)]

        x_sync = ctx.enter_context(nc.semaphore("x_sync"))
        x_sc = ctx.enter_context(nc.semaphore("x_sc"))
        x_gp = ctx.enter_context(nc.semaphore("x_gp"))
        o_sync = ctx.enter_context(nc.semaphore("o_sync"))
        o_sc = ctx.enter_context(nc.semaphore("o_sc"))
        o_gp = ctx.enter_context(nc.semaphore("o_gp"))
        mm_sem = ctx.enter_context(nc.semaphore("mm_sem"))
        v_sem = ctx.enter_context(nc.semaphore("v_sem"))

        block = ctx.enter_context(nc.Block(no_gpsimd_drain=True))

        def img_piece(dma, b, r_lo, r_hi, sem):
            lo, hi = b * XF + r_lo * WP, b * XF + r_hi * WP
            dma.dma_start(out=xbuf[:, lo:hi], in_=xp[:, lo:hi]).then_inc(sem, 16)

        def out_piece(dma, b, c_lo, c_hi, sem):
            dma.dma_start(
                out=op[b][:, c_lo * CW:c_hi * CW],
                in_=o_slots[SLOT[b]][:, c_lo * CW:c_hi * CW],
            ).then_inc(sem, 16)

        @block.sync
        def _(sync):
            # Few, big DMAs: every dma_start pays ~1.5-2us of serial
            # descriptor generation on its ring before data moves (measured),
            # so fine-grained pieces starve the ring.  Rings are issued in
            # need-order and later inputs are sem-gated behind earlier ones so
            # they cannot steal HBM bandwidth from the critical early pieces.
            img_piece(sync, 0, 0, 26, x_sync)       # img0 rows 0-25 (g0-2)
            img_piece(sync, 0, 26, HP, x_sync)      # img0 rows 26-58 (g3-6)
            sync.wait_ge(v_sem, NCHUNK)
            out_piece(sync, 0, 0, NCHUNK, o_sync)   # out0 whole (sem: slot0
            sync.wait_ge(v_sem, 3 * NCHUNK)         # is reused by img3)
            out_piece(sync, 2, 0, NCHUNK, o_sync)   # out2 whole
            sync.wait_ge(o_sync, 32)

        @block.scalar
        def _(scalar):
            scalar.dma_start(out=w_tile[:], in_=wp[:]).then_inc(x_sc, 16)
            scalar.wait_ge(x_sync, 16)              # let img0-front go first
            img_piece(scalar, 1, 0, HP, x_sc)       # img1 whole (g7)
            scalar.wait_ge(v_sem, 2 * NCHUNK)
            out_piece(scalar, 1, 0, NCHUNK, o_sc)   # out1 whole
            scalar.wait_ge(o_sc, 16)

        @block.gpsimd
        def _(gpsimd):
            # The whole img3 tail rides this one SWDGE queue, kept otherwise
            # empty: the first gated piece pays the full ~3us wait->data
            # latency, but each later piece's descriptor-gen pipelines
            # behind its predecessor's transfer (measured +0.6us/piece), so
            # the last piece lands far earlier than on any idle ring.  All
            # completions are awaited before the block ends (a
            # fire-and-forget variant that let the teardown cover the last
            # transfers was ~2us faster but NaN'd on a cold first run).
            gpsimd.wait_ge(x_sync, 32)              # let img0/img1 go first
            img_piece(gpsimd, 2, 0, HP, x_gp)       # img2 whole (g14)
            img_piece(gpsimd, 3, 0, HP, x_gp)       # img3 whole (g21)
            gpsimd.wait_ge(v_sem, 3 * NCHUNK + 1)
            out_piece(gpsimd, 3, 0, 1, o_gp)        # out3 c0 (starts the
            gpsimd.wait_ge(v_sem, 3 * NCHUNK + 3)   # chain early: its ~3.4us
            out_piece(gpsimd, 3, 1, 3, o_gp)        # latency hides under the
            gpsimd.wait_ge(v_sem, 3 * NCHUNK + 5)   # last ~4us of compute)
            out_piece(gpsimd, 3, 3, 5, o_gp)
            gpsimd.wait_ge(v_sem, 3 * NCHUNK + 6)
            out_piece(gpsimd, 3, 5, 6, o_gp)        # out3 c5
            gpsimd.wait_ge(v_sem, len(CHUNKS))      # all img3 tail copies
            out_piece(gpsimd, 3, 6, 7, o_gp)        # out3 c6
            gpsimd.wait_ge(o_gp, 80)

        @block.tensor
        def _(tensor):
            # dummy matmuls on garbage data: continuous PE activity from the
            # earliest possible moment lifts the HAM clock gate (free-running
            # 3.4us activity window, 1.2 -> 2.4GHz); any idle gap before real
            # work disqualifies the window and delays the boost (measured
            # +5us), so the dummies must bridge to the first gated matmul.
            for _ in range(12):
                tensor.matmul(
                    psums[NPSUM - 1][0:32, :NFREE],
                    w_tile[:, 0:32],
                    xbuf[:, 0:NFREE],
                    start=True, stop=True,
                    tile_position=(0, 0),
                )
            tensor.wait_ge(x_sc, 16)        # weights
            tensor.wait_ge(x_sync, 16)      # img0 rows 0-9
            # (global chunk -> input-piece semaphore threshold) gates
            gates = {3: (x_sync, 32), 7: (x_sc, 32),
                     14: (x_gp, 16), 21: (x_gp, 32)}
            for g, (b, r0, nr) in enumerate(CHUNKS):
                if g in gates:
                    tensor.wait_ge(*gates[g])
                if g >= NPSUM:
                    # psum bank g%NPSUM free once chunk g-NPSUM was copied
                    tensor.wait_ge(v_sem, g - NPSUM + 1)
                bank = psums[g % NPSUM]
                nfree = nr * WP
                for j in range(4):
                    for grp in range(4):
                        t = CFG[grp][j]
                        kr, kc = (t - 1) // 3, (t - 1) % 3
                        off = b * XF + (r0 + kr) * WP + kc
                        idx = grp * 4 + j
                        mm = tensor.matmul(
                            bank[32 * grp:32 * (grp + 1), :nfree],
                            w_tile[:, idx * 32:(idx + 1) * 32],
                            xbuf[:, off:off + nfree],
                            start=(j == 0),
                            stop=(j == 3),
                            tile_position=(0, 32 * grp),
                        )
                mm.then_inc(mm_sem, 1)
        @block.vector
        def _(vector):
            for g, (b, r0, nr) in enumerate(CHUNKS):
                if b == 3 and r0 == 0:
                    vector.wait_ge(o_sync, 16)   # out0 done -> slot0 free
                vector.wait_ge(mm_sem, g + 1)
                src = psums[g % NPSUM][:, :nr * WP].rearrange(
                    "p (r w) -> p r w", w=WP)[:, :, :W]
                dst = o_slots[SLOT[b]][:, r0 * W:(r0 + nr) * W].rearrange(
                    "p (r w) -> p r w", w=W)
                vector.tensor_copy(out=dst, in_=src).then_inc(v_sem, 1)

    return nc


_NC_CACHE = None


def _get_nc():
    global _NC_CACHE
    if _NC_CACHE is None:
        _NC_CACHE = _build_nc()
    return _NC_CACHE


def _prep_weights(weights):
    """(32, 2048) grouped-sparse -> 16 [ic=128, oc=32] fp16 lhsT blocks."""
    w16 = np.zeros((C, 16 * 32), np.float32)
    for g, taps in enumerate(CFG):
        blk = np.asarray(weights[:, g * 512:(g + 1) * 512], np.float32)
        blk = blk.reshape(32, C, 4)  # [oc_in_group, ic, tap_j]
        for j in range(4):
            idx = g * 4 + j
            w16[:, idx * 32:(idx + 1) * 32] = blk[:, :, j].T
    return np.ascontiguousarray(w16.astype(np.float16))


def _prep_x(x):
    """(32,128,56,56) f32 -> per-core channel-major padded fp16 shards."""
    xpad = np.zeros((B, C, HP, WP), np.float16)
    xpad[:, :, 1:H + 1, 1:W + 1] = x.astype(np.float16)
    xs = xpad.reshape(NCORES, BPC, C, XF)
    # (core, b, c, f) -> (core, c, b*f)
    xs = np.ascontiguousarray(xs.transpose(0, 2, 1, 3)).reshape(NCORES, C, BPC * XF)
    return xs


def kernel(x, weights):
    x = np.asarray(x, np.float32)
    weights = np.asarray(weights, np.float32)

    xs = _prep_x(x)
    wflat = _prep_weights(weights)

    nc = _get_nc()
    in_maps = [{"x": xs[i], "w": wflat} for i in range(NCORES)]
    res = run_bass_kernel_spmd(nc, in_maps, core_ids=list(range(NCORES)))
    return np.concatenate(
        [res.results[i]["out"].astype(np.float32).reshape(BPC, C, H, W)
         for i in range(NCORES)],
        axis=0,
    )

